# revision 23
# baseline (speedup 1.0000x reference)
"""Trainium2 Bass kernel for causal self-attention (nn_CausalSelfAttention).

Problem (hardcoded):
    x:     [1, 4096, 1024] f32
    w_qkv: [1024, 3072] f32, b_qkv: [3072] f32
    w_out: [1024, 1024] f32, b_out: [1024] f32
    16 heads, head_dim 64, causal softmax attention.

Sharding: tensor-parallel over heads. 8 cores x 2 heads each. Each core
computes QKV for its heads, T^2 causal attention, and a partial output
projection; host sums the 8 partial projections (the all-reduce) and adds
biases.

Math notes (exact simplifications):
  - b_k drops out: softmax is shift-invariant along keys.
  - b_v reduces to a host-side constant row b_v @ w_out (attn rows sum to 1).
  - b_q applied on-device as a per-partition bias when copying Q^T from PSUM.
  - Softmax denominators commute with the output projection; normalization
    happens once per tile right before the projection.

Performance notes (vs the f32r baseline):
  - All matmul operands are fp16 (1 cycle/row on the PE like f32r, but no
    4x penalty for moving dims < 256 on the diagonal chunks); PSUM stays
    f32. Input/output DMA halves (x^T, w, y in fp16).
  - DMA issue order: (w chunk, x tile0 chunk) pairs first so the first QKV
    matmul can start ~1 MB into the stream instead of after all constants.
  - Attention inner loop is software-pipelined: scores for chunk g+1 issue
    before the AV matmuls of chunk g, so the exp (Scalar engine) latency
    hides behind PE work. QKV for tile j+1 and the output projection of
    tile j-1 are interleaved into tile j's chunk stream as filler PE work.
  - Causal-mask multiplies and the PSUM->SBUF output-projection copies run
    on GpSimd (otherwise idle), keeping Vector free for the QKV copies.
  - The two per-head denominator-broadcast matmuls are merged into one
    (stationary [2,128] selector, moving [2,TQ] stacked denominators).
"""

import numpy as np
import ml_dtypes

T = 4096
E = 1024
NCORES = 8
D = 64  # head dim
TQ = 512  # query tile (8 tiles)
NJ = T // TQ

_CACHE = {}

# Results of the last SPMD run (exec_time_ns etc.), for the local test harness.
LAST_RESULTS = None


def _build():
    import concourse.bacc as bacc
    import concourse.tile as tile
    import concourse.mybir as mybir

    f32 = mybir.dt.float32
    f32r = mybir.dt.float32r
    f16 = mybir.dt.float16
    EXP = mybir.ActivationFunctionType.Exp

    nc = bacc.Bacc("TRN2", target_bir_lowering=False, debug=False)

    xT = nc.dram_tensor("xT", [E, T], f16, kind="ExternalInput").ap()
    # per-core slice of w_qkv: cols [q(128) | k(128) | v(128)] for this core's
    # two heads
    wqkv = nc.dram_tensor("wqkv", [E, 384], f16, kind="ExternalInput").ap()
    bq = nc.dram_tensor("bq", [128], f32, kind="ExternalInput").ap()
    wo = nc.dram_tensor("wo", [128, E], f16, kind="ExternalInput").ap()
    # [sel_h0(128) | sel_h1(128)] selector row for the denominator broadcast
    sel_dram = nc.dram_tensor("sel", [1, 256], f32r, kind="ExternalInput").ap()
    mask_dram = nc.dram_tensor("mask", [128, 128], f16, kind="ExternalInput").ap()
    ident_dram = nc.dram_tensor("ident", [128, 128], f16, kind="ExternalInput").ap()
    y = nc.dram_tensor("y", [T, E], f16, kind="ExternalOutput").ap()

    with tile.TileContext(nc) as tc:
        with (
            tc.tile_pool(name="consts", bufs=1) as consts,
            tc.tile_pool(name="w", bufs=8) as wpool,
            tc.tile_pool(name="xt", bufs=16) as xtp,
            tc.tile_pool(name="qt", bufs=2) as qtp,
            tc.tile_pool(name="kt", bufs=NJ) as ktp,
            tc.tile_pool(name="v", bufs=NJ) as vp,
            tc.tile_pool(name="vts", bufs=2) as vtsp,
            tc.tile_pool(name="expst", bufs=6) as exp_p,
            tc.tile_pool(name="otn", bufs=8) as otnp,
            tc.tile_pool(name="bb", bufs=2) as bbp,
            tc.tile_pool(name="rd", bufs=4) as rdp,
            tc.tile_pool(name="ysb", bufs=3) as ysp,
            tc.tile_pool(name="mm_ps", bufs=2, space="PSUM") as mmp,
            tc.tile_pool(name="st_ps", bufs=2, space="PSUM") as stp,
            tc.tile_pool(name="op_ps", bufs=2, space="PSUM") as opp,
        ):
            # ---- DMA priority order: first w/x pairs so the PE can start.
            # The very first pair is split into partition halves so two DMA
            # engines carry each and the first matmul starts sooner.
            w_sb = []
            xts0 = []
            for e in range(8):
                w = wpool.tile([128, 384], f16)
                if e == 0:
                    nc.sync.dma_start(w[0:64, :], wqkv[0:64, :])
                    nc.sync.dma_start(w[64:128, :], wqkv[64:128, :])
                else:
                    nc.sync.dma_start(w[:], wqkv[128 * e : 128 * (e + 1), :])
                w_sb.append(w)
                xt = xtp.tile([128, TQ], f16)
                if e == 0:
                    nc.sync.dma_start(xt[0:64, :], xT[0:64, 0:TQ])
                    nc.sync.dma_start(xt[64:128, :], xT[64:128, 0:TQ])
                else:
                    nc.sync.dma_start(xt[:], xT[128 * e : 128 * (e + 1), 0:TQ])
                xts0.append(xt)
            bq_sb = consts.tile([128, 1], f32)
            nc.sync.dma_start(bq_sb[:, 0], bq[:])
            mask = consts.tile([128, 128], f16)  # 1 where tq >= tk else 0
            nc.sync.dma_start(mask[:], mask_dram[:])
            ident = consts.tile([128, 128], f16)
            nc.sync.dma_start(ident[:], ident_dram[:])
            sel = consts.tile([1, 256], f32r)
            nc.sync.dma_start(sel[:], sel_dram[:])
            wo_sb = consts.tile([128, E], f16)
            nc.sync.dma_start(wo_sb[:], wo[:])

            def load_xt(j):
                xts = []
                for e in range(8):
                    xt = xtp.tile([128, TQ], f16, name="xt")
                    nc.sync.dma_start(
                        xt[:], xT[128 * e : 128 * (e + 1), TQ * j : TQ * j + TQ]
                    )
                    xts.append(xt)
                return xts

            def qkv_units(xts):
                """Filler units computing Q^T/K^T/V for one tile from its x^T
                tiles. Returns (q_units, kv_units, out): the Q part runs during
                the previous tile (qt is needed at tile start); the K/V parts
                run inside the tile's own early chunks (kt/vt are only needed
                by its diagonal chunks at the end), which feeds the PE in the
                otherwise scalar-bound attention stretches."""
                out = {}
                ps = {}

                def mk_mm(which, col0, e0):
                    def run():
                        if e0 == 0:
                            ps[which] = mmp.tile([128, TQ], f32, tag="mm", name=f"ps_{which}")
                        for e in (e0, e0 + 1, e0 + 2, e0 + 3):
                            nc.tensor.matmul(
                                ps[which][:],
                                w_sb[e][:, col0 : col0 + 128],
                                xts[e][:],
                                start=(e == 0),
                                stop=(e == 7),
                            )
                    return run

                def fin_q():
                    qt = qtp.tile([128, TQ], f16, name="qt")
                    nc.vector.tensor_scalar_add(qt[:], ps["q"][:], bq_sb[:, 0:1])
                    out["qt"] = qt

                def fin_k():
                    kt = ktp.tile([128, TQ], f16, name="kt")
                    nc.vector.tensor_copy(kt[:], ps["k"][:])
                    out["kt"] = kt
                    kt_tiles.append(kt)

                def fin_v():
                    vts = vtsp.tile([128, TQ], f16, name="vts")
                    nc.vector.tensor_copy(vts[:], ps["v"][:])
                    ps["vts"] = vts
                    vt = vp.tile([128, 4 * 130], f16, name="vt")
                    nc.vector.memset(
                        vt.rearrange("p (c w) -> p c w", w=130)[:, :, 64::65], 1.0
                    )
                    out["vt"] = vt
                    v_tiles.append(vt)

                def mk_tr(c):
                    def run():
                        ps_tr = mmp.tile([128, 128], f16, tag="mm", name="ps_tr")
                        nc.tensor.transpose(
                            ps_tr[:], ps["vts"][:, 128 * c : 128 * (c + 1)], ident[:]
                        )
                        vt = out["vt"]
                        nc.vector.tensor_copy(
                            vt[:, 130 * c : 130 * c + 64], ps_tr[:, 0:64]
                        )
                        nc.vector.tensor_copy(
                            vt[:, 130 * c + 65 : 130 * c + 129], ps_tr[:, 64:128]
                        )
                    return run

                q_units = [mk_mm("q", 0, 0), mk_mm("q", 0, 4), fin_q]
                kv_units = [mk_mm("k", 128, 0), mk_mm("k", 128, 4), fin_k]
                kv_units += [mk_mm("v", 256, 0), mk_mm("v", 256, 4), fin_v]
                kv_units += [mk_tr(c) for c in range(4)]
                return q_units, kv_units, out

            COPYF = mybir.ActivationFunctionType.Copy

            def proj_chunk(otn, t0, c, split=False):
                """One 128-token chunk of the output projection. With
                split=True (kernel tail) the two PSUM->SBUF casts alternate
                between Vector and Scalar and each half DMAs out on its own,
                so the drain pipeline overlaps."""
                ys = ysp.tile([128, E], f16, tag="ys", name=f"ys_{t0}_{c}")
                for half in range(2):
                    yp = mmp.tile(
                        [128, 512], f32, tag="mm", name=f"yp_{t0}_{c}_{half}"
                    )
                    nc.tensor.matmul(
                        yp[:],
                        otn[:, 128 * c : 128 * (c + 1)],
                        wo_sb[:, 512 * half : 512 * (half + 1)],
                        start=True,
                        stop=True,
                    )
                    if split and half == 1:
                        nc.scalar.activation(
                            ys[:, 512 * half : 512 * (half + 1)], yp[:], COPYF
                        )
                    else:
                        nc.vector.tensor_copy(
                            ys[:, 512 * half : 512 * (half + 1)], yp[:]
                        )
                    if split:
                        nc.sync.dma_start(
                            y[
                                t0 + 128 * c : t0 + 128 * (c + 1),
                                512 * half : 512 * (half + 1),
                            ],
                            ys[:, 512 * half : 512 * (half + 1)],
                        )
                if not split:
                    nc.sync.dma_start(
                        y[t0 + 128 * c : t0 + 128 * (c + 1), :], ys[:]
                    )

            def outproj_units(ops_d):
                """Partial output projection for a finished tile; reads the
                normalized O^T lazily so it can be scheduled before the
                norm_unit closure has run at emission time."""

                def mk(c):
                    def run():
                        otn, t0 = ops_d["otn"]
                        proj_chunk(otn, t0, c)
                    return run

                return [mk(c) for c in range(4)]

            def norm_unit(ops, t0):
                """Denominator broadcast -> reciprocal -> normalized O^T.
                Runs as an early filler of the NEXT tile so the PE never
                waits on the reciprocal chain. The rd copies are emitted at
                the owning tile's end (vector only, no PE)."""
                def run():
                    rd0, rd1 = ops["rd"]
                    bps = mmp.tile([128, TQ], f32, tag="mm", name=f"bps_{t0}")
                    nc.tensor.matmul(
                        bps[:], sel[0:1, 0:128], rd0[:], start=True, stop=False
                    )
                    nc.tensor.matmul(
                        bps[:], sel[0:1, 128:256], rd1[:], start=False, stop=True
                    )
                    bb = bbp.tile([128, TQ], f32, tag="bb", name=f"bb_{t0}")
                    nc.vector.reciprocal_approx_fast(bb[:], bps[:])
                    otn = otnp.tile([128, TQ], f16, tag="otn", name=f"otn_{t0}")
                    nc.vector.tensor_mul(
                        otn[0:64, :], ops[0][0:64, :], bb[0:64, :]
                    )
                    nc.vector.tensor_mul(
                        otn[64:128, :], ops[1][0:64, :], bb[64:128, :]
                    )
                    ops["otn"] = (otn, t0)
                return run

            # ---- main pipeline ----
            kt_tiles = []
            v_tiles = []

            q0, kv0, out0 = qkv_units(xts0)
            for u in q0 + kv0:
                u()
            cur = out0
            kv_pending = []  # K/V units of the current tile j (early chunks)
            # output projections are deferred to the late, scalar-bound tiles
            OPSCHED = {5: [0], 6: [1, 2], 7: [3, 4, 5, 6]}
            tile_ops = {}  # j -> ops dict (op tiles, rd, otn)

            for j in range(NJ):
                t0 = TQ * j
                qt = cur["qt"]

                nchunks = 4 * j + 4
                slots = {}

                def place(units, lo, hi):
                    # spread units over chunk indices [lo, hi] (inclusive)
                    if not units:
                        return
                    lo = max(0, min(lo, nchunks - 1))
                    hi = max(lo, min(hi, nchunks - 1))
                    span = hi - lo + 1
                    for i, u in enumerate(units):
                        g = lo + (i * span) // len(units)
                        slots.setdefault(g, []).append(u)

                # normalization of the previous tile: first chunk
                if j > 0:
                    place([norm_unit(tile_ops[j - 1], TQ * (j - 1))], 0, 0)
                # this tile's K/V: must land before the diagonal chunks (4j)
                place(kv_pending, 0, 4 * j - 1)
                fillers = []
                nxt = None
                if j + 1 < NJ:
                    xts = load_xt(j + 1)
                    qu, kvu, nxt = qkv_units(xts)
                    fillers += qu
                    kv_pending = kvu
                else:
                    kv_pending = []
                for i in OPSCHED.get(j, ()):
                    fillers += outproj_units(tile_ops[i])
                place(fillers, 2, nchunks - 1)

                class OpsDict(dict):
                    pass

                ops = OpsDict()
                ops[0] = opp.tile([65, TQ], f32, tag="op", name="op0")
                ops[1] = opp.tile([65, TQ], f32, tag="op", name="op1")
                tile_ops[j] = ops

                def emit_scores(g):
                    jj, c = divmod(g, 4)
                    r = g - 4 * j
                    col0 = 128 * r if r >= 0 else 0
                    st = stp.tile([128, 2 * TQ], f32, tag="st")
                    for h in range(2):
                        nc.tensor.matmul(
                            st[:, TQ * h + col0 : TQ * h + TQ],
                            kt_tiles[jj][64 * h : 64 * h + 64, 128 * c : 128 * (c + 1)],
                            qt[64 * h : 64 * h + 64, col0:TQ],
                            start=True,
                            stop=True,
                        )
                    return st, col0

                def emit_exp(st, col0, g):
                    ex = exp_p.tile([128, 2 * TQ], f16, tag="ex")
                    st3 = st.rearrange("p (h n) -> p h n", h=2)
                    ex3 = ex.rearrange("p (h n) -> p h n", h=2)
                    nc.scalar.activation(
                        ex3[:, :, col0:TQ], st3[:, :, col0:TQ], EXP, scale=0.125
                    )
                    r = g - 4 * j
                    if r >= 0:
                        for h in range(2):
                            nc.gpsimd.tensor_mul(
                                ex[:, TQ * h + col0 : TQ * h + col0 + 128],
                                ex[:, TQ * h + col0 : TQ * h + col0 + 128],
                                mask[:],
                            )
                    return ex

                def emit_av(ex, col0, g):
                    jj, c = divmod(g, 4)
                    for h in range(2):
                        nc.tensor.matmul(
                            ops[h][:, col0:TQ],
                            v_tiles[jj][:, 130 * c + 65 * h : 130 * c + 65 * h + 65],
                            ex[:, TQ * h + col0 : TQ * h + TQ],
                            start=(g == 0),
                            stop=(g == nchunks - 1),
                            skip_group_check=True,
                        )

                # software-pipelined chunk stream: S(g+1) issues before A(g)
                pend = None
                for g in range(nchunks):
                    st, col0 = emit_scores(g)
                    ex = emit_exp(st, col0, g)
                    for u in slots.get(g, ()):
                        u()
                    if pend is not None:
                        emit_av(*pend)
                    pend = (ex, col0, g)
                emit_av(*pend)

                # grab the denominator rows; the rest of the normalization
                # runs as the next tile's first filler
                rd0 = rdp.tile([1, TQ], f32r, tag="rd", name="rd0")
                rd1 = rdp.tile([1, TQ], f32r, tag="rd", name="rd1")
                with nc.allow_low_precision(reason="f32r rounding of denom"):
                    nc.vector.tensor_copy(rd0[:], ops[0][64:65, :])
                    nc.vector.tensor_copy(rd1[:], ops[1][64:65, :])
                ops["rd"] = (rd0, rd1)
                cur = nxt

            # ---- tail: last tile's normalization + projection, pipelined
            # per 128-token chunk so casts/DMAs overlap the matmuls
            ops = tile_ops[NJ - 1]
            t0 = TQ * (NJ - 1)
            rd0, rd1 = ops["rd"]
            bps = mmp.tile([128, TQ], f32, tag="mm", name="bps_last")
            nc.tensor.matmul(bps[:], sel[0:1, 0:128], rd0[:], start=True, stop=False)
            nc.tensor.matmul(
                bps[:], sel[0:1, 128:256], rd1[:], start=False, stop=True
            )
            bb = bbp.tile([128, TQ], f32, tag="bb", name="bb_last")
            nc.vector.reciprocal_approx_fast(bb[:], bps[:])
            otn = otnp.tile([128, TQ], f16, tag="otn", name="otn_last")
            for c in range(4):
                sl = slice(128 * c, 128 * (c + 1))
                nc.vector.tensor_mul(otn[0:64, sl], ops[0][0:64, sl], bb[0:64, sl])
                nc.vector.tensor_mul(
                    otn[64:128, sl], ops[1][0:64, sl], bb[64:128, sl]
                )
                proj_chunk(otn, t0, c, split=True)

    nc.compile()
    return nc


def _prep_inputs(x, w_qkv, b_qkv, w_out, b_out):
    x = np.asarray(x, dtype=np.float32).reshape(T, E)
    w_qkv = np.asarray(w_qkv, dtype=np.float32)
    b_qkv = np.asarray(b_qkv, dtype=np.float32)
    w_out = np.asarray(w_out, dtype=np.float32)
    b_out = np.asarray(b_out, dtype=np.float32)

    xT = np.ascontiguousarray(x.T).astype(np.float16)
    mask = np.triu(np.ones((128, 128), dtype=np.float16))
    ident = np.eye(128, dtype=np.float16)
    sel = np.zeros((1, 256), dtype=np.float32)
    sel[0, 0:64] = 1.0
    sel[0, 192:256] = 1.0

    in_maps = []
    for cidx in range(NCORES):
        lo, hi = 128 * cidx, 128 * (cidx + 1)
        wq = w_qkv[:, lo:hi]
        wk = w_qkv[:, E + lo : E + hi]
        wv = w_qkv[:, 2 * E + lo : 2 * E + hi]
        wqkv_c = np.ascontiguousarray(
            np.concatenate([wq, wk, wv], axis=1)
        ).astype(np.float16)
        in_maps.append(
            {
                "xT": xT,
                "wqkv": wqkv_c,
                "bq": np.ascontiguousarray(b_qkv[lo:hi]),
                "wo": np.ascontiguousarray(w_out[lo:hi, :]).astype(np.float16),
                "sel": sel,
                "mask": mask,
                "ident": ident,
            }
        )
    # host-side constant: b_out plus the exact b_v contribution
    b_v = b_qkv[2 * E : 3 * E]
    const_row = b_out + b_v @ w_out
    return in_maps, const_row


def kernel(x, w_qkv, b_qkv, w_out, b_out):
    global LAST_RESULTS
    from concourse.bass_utils import run_bass_kernel_spmd

    if "nc" not in _CACHE:
        _CACHE["nc"] = _build()
    nc = _CACHE["nc"]

    in_maps, const_row = _prep_inputs(x, w_qkv, b_qkv, w_out, b_out)
    res = run_bass_kernel_spmd(nc, in_maps, core_ids=list(range(NCORES)))
    LAST_RESULTS = res

    out = np.zeros((T, E), dtype=np.float32)
    for r in res.results:
        out += r["y"].astype(np.float32)
    out += const_row[None, :].astype(np.float32)
    return out.reshape(1, T, E)


# revision 24
# speedup vs baseline: 1.0209x; 1.0209x over previous
"""Trainium2 Bass kernel for causal self-attention (nn_CausalSelfAttention).

Problem (hardcoded):
    x:     [1, 4096, 1024] f32
    w_qkv: [1024, 3072] f32, b_qkv: [3072] f32
    w_out: [1024, 1024] f32, b_out: [1024] f32
    16 heads, head_dim 64, causal softmax attention.

Sharding: tensor-parallel over heads. 8 cores x 2 heads each. Each core
computes QKV for its heads, T^2 causal attention, and a partial output
projection; host sums the 8 partial projections (the all-reduce) and adds
biases.

Math notes (exact simplifications):
  - b_k drops out: softmax is shift-invariant along keys.
  - b_v reduces to a host-side constant row b_v @ w_out (attn rows sum to 1).
  - b_q applied on-device as a per-partition bias when copying Q^T from PSUM.
  - Softmax denominators commute with the output projection; normalization
    happens once per tile right before the projection.

Performance notes (vs the f32r baseline):
  - All matmul operands are fp16 (1 cycle/row on the PE like f32r, but no
    4x penalty for moving dims < 256 on the diagonal chunks); PSUM stays
    f32. Input/output DMA halves (x^T, w, y in fp16).
  - DMA issue order: (w chunk, x tile0 chunk) pairs first so the first QKV
    matmul can start ~1 MB into the stream instead of after all constants.
  - Attention inner loop is software-pipelined: scores for chunk g+1 issue
    before the AV matmuls of chunk g, so the exp (Scalar engine) latency
    hides behind PE work. QKV for tile j+1 and the output projection of
    tile j-1 are interleaved into tile j's chunk stream as filler PE work.
  - Causal-mask multiplies and the PSUM->SBUF output-projection copies run
    on GpSimd (otherwise idle), keeping Vector free for the QKV copies.
  - The two per-head denominator-broadcast matmuls are merged into one
    (stationary [2,128] selector, moving [2,TQ] stacked denominators).
"""

import numpy as np
import ml_dtypes

T = 4096
E = 1024
NCORES = 8
D = 64  # head dim
TQ = 512  # query tile (8 tiles)
NJ = T // TQ

_CACHE = {}

# Results of the last SPMD run (exec_time_ns etc.), for the local test harness.
LAST_RESULTS = None


def _build():
    import concourse.bacc as bacc
    import concourse.tile as tile
    import concourse.mybir as mybir

    f32 = mybir.dt.float32
    f32r = mybir.dt.float32r
    f16 = mybir.dt.float16
    EXP = mybir.ActivationFunctionType.Exp

    nc = bacc.Bacc("TRN2", target_bir_lowering=False, debug=False)

    xT = nc.dram_tensor("xT", [E, T], f16, kind="ExternalInput").ap()
    # per-core slice of w_qkv: cols [q(128) | k(128) | v(128)] for this core's
    # two heads
    wqkv = nc.dram_tensor("wqkv", [E, 384], f16, kind="ExternalInput").ap()
    bq = nc.dram_tensor("bq", [128], f32, kind="ExternalInput").ap()
    wo = nc.dram_tensor("wo", [128, E], f16, kind="ExternalInput").ap()
    # [sel_h0(128) | sel_h1(128)] selector row for the denominator broadcast
    sel_dram = nc.dram_tensor("sel", [1, 256], f32r, kind="ExternalInput").ap()
    mask_dram = nc.dram_tensor("mask", [128, 128], f16, kind="ExternalInput").ap()
    ident_dram = nc.dram_tensor("ident", [128, 128], f16, kind="ExternalInput").ap()
    y = nc.dram_tensor("y", [T, E], f16, kind="ExternalOutput").ap()

    with tile.TileContext(nc) as tc:
        with (
            tc.tile_pool(name="consts", bufs=1) as consts,
            tc.tile_pool(name="w", bufs=8) as wpool,
            tc.tile_pool(name="xt", bufs=16) as xtp,
            tc.tile_pool(name="qt", bufs=2) as qtp,
            tc.tile_pool(name="kt", bufs=NJ) as ktp,
            tc.tile_pool(name="v", bufs=NJ) as vp,
            tc.tile_pool(name="vts", bufs=2) as vtsp,
            tc.tile_pool(name="expst", bufs=6) as exp_p,
            tc.tile_pool(name="otn", bufs=8) as otnp,
            tc.tile_pool(name="bb", bufs=2) as bbp,
            tc.tile_pool(name="rd", bufs=4) as rdp,
            tc.tile_pool(name="ysb", bufs=3) as ysp,
            tc.tile_pool(name="mm_ps", bufs=2, space="PSUM") as mmp,
            tc.tile_pool(name="st_ps", bufs=2, space="PSUM") as stp,
            tc.tile_pool(name="op_ps", bufs=2, space="PSUM") as opp,
        ):
            # ---- DMA priority order: first w/x pairs so the PE can start.
            # The very first pair is split into partition halves so two DMA
            # engines carry each and the first matmul starts sooner.
            w_sb = []
            xts0 = []
            for e in range(8):
                w = wpool.tile([128, 384], f16)
                if e == 0:
                    nc.sync.dma_start(w[0:64, :], wqkv[0:64, :])
                    nc.sync.dma_start(w[64:128, :], wqkv[64:128, :])
                else:
                    nc.sync.dma_start(w[:], wqkv[128 * e : 128 * (e + 1), :])
                w_sb.append(w)
                xt = xtp.tile([128, TQ], f16)
                if e == 0:
                    nc.sync.dma_start(xt[0:64, :], xT[0:64, 0:TQ])
                    nc.sync.dma_start(xt[64:128, :], xT[64:128, 0:TQ])
                else:
                    nc.sync.dma_start(xt[:], xT[128 * e : 128 * (e + 1), 0:TQ])
                xts0.append(xt)
            bq_sb = consts.tile([128, 1], f32)
            nc.sync.dma_start(bq_sb[:, 0], bq[:])
            mask = consts.tile([128, 128], f16)  # 1 where tq >= tk else 0
            nc.sync.dma_start(mask[:], mask_dram[:])
            ident = consts.tile([128, 128], f16)
            nc.sync.dma_start(ident[:], ident_dram[:])
            sel = consts.tile([1, 256], f32r)
            nc.sync.dma_start(sel[:], sel_dram[:])
            wo_sb = consts.tile([128, E], f16)
            nc.sync.dma_start(wo_sb[:], wo[:])

            def load_xt(j):
                xts = []
                for e in range(8):
                    xt = xtp.tile([128, TQ], f16, name="xt")
                    nc.sync.dma_start(
                        xt[:], xT[128 * e : 128 * (e + 1), TQ * j : TQ * j + TQ]
                    )
                    xts.append(xt)
                return xts

            def qkv_units(xts):
                """Filler units computing Q^T/K^T/V for one tile from its x^T
                tiles. Returns (q_units, kv_units, out): the Q part runs during
                the previous tile (qt is needed at tile start); the K/V parts
                run inside the tile's own early chunks (kt/vt are only needed
                by its diagonal chunks at the end), which feeds the PE in the
                otherwise scalar-bound attention stretches."""
                out = {}
                ps = {}

                def mk_mm(which, col0, e):
                    def run():
                        if e == 0:
                            ps[which] = mmp.tile(
                                [128, TQ], f32, tag="mm", name=f"ps_{which}"
                            )
                        nc.tensor.matmul(
                            ps[which][:],
                            w_sb[e][:, col0 : col0 + 128],
                            xts[e][:],
                            start=(e == 0),
                            stop=(e == 7),
                        )
                    return run

                def fin_q():
                    qt = qtp.tile([128, TQ], f16, name="qt")
                    nc.vector.tensor_scalar_add(qt[:], ps["q"][:], bq_sb[:, 0:1])
                    out["qt"] = qt

                def fin_k():
                    kt = ktp.tile([128, TQ], f16, name="kt")
                    nc.vector.tensor_copy(kt[:], ps["k"][:])
                    out["kt"] = kt
                    kt_tiles.append(kt)

                def fin_v():
                    vts = vtsp.tile([128, TQ], f16, name="vts")
                    nc.vector.tensor_copy(vts[:], ps["v"][:])
                    ps["vts"] = vts
                    vt = vp.tile([128, 4 * 130], f16, name="vt")
                    nc.vector.memset(
                        vt.rearrange("p (c w) -> p c w", w=130)[:, :, 64::65], 1.0
                    )
                    out["vt"] = vt
                    v_tiles.append(vt)

                def mk_tr(c):
                    def run():
                        ps_tr = mmp.tile([128, 128], f16, tag="mm", name="ps_tr")
                        nc.tensor.transpose(
                            ps_tr[:], ps["vts"][:, 128 * c : 128 * (c + 1)], ident[:]
                        )
                        vt = out["vt"]
                        nc.vector.tensor_copy(
                            vt[:, 130 * c : 130 * c + 64], ps_tr[:, 0:64]
                        )
                        nc.vector.tensor_copy(
                            vt[:, 130 * c + 65 : 130 * c + 129], ps_tr[:, 64:128]
                        )
                    return run

                q_units = [mk_mm("q", 0, e) for e in range(8)] + [fin_q]
                kv_units = [mk_mm("k", 128, e) for e in range(8)] + [fin_k]
                kv_units += [mk_mm("v", 256, e) for e in range(8)] + [fin_v]
                kv_units += [mk_tr(c) for c in range(4)]
                return q_units, kv_units, out

            COPYF = mybir.ActivationFunctionType.Copy

            def proj_chunk(otn, t0, c, split=False):
                """One 128-token chunk of the output projection. With
                split=True (kernel tail) the two PSUM->SBUF casts alternate
                between Vector and Scalar and each half DMAs out on its own,
                so the drain pipeline overlaps."""
                ys = ysp.tile([128, E], f16, tag="ys", name=f"ys_{t0}_{c}")
                for half in range(2):
                    yp = mmp.tile(
                        [128, 512], f32, tag="mm", name=f"yp_{t0}_{c}_{half}"
                    )
                    nc.tensor.matmul(
                        yp[:],
                        otn[:, 128 * c : 128 * (c + 1)],
                        wo_sb[:, 512 * half : 512 * (half + 1)],
                        start=True,
                        stop=True,
                    )
                    if split and half == 1:
                        nc.scalar.activation(
                            ys[:, 512 * half : 512 * (half + 1)], yp[:], COPYF
                        )
                    else:
                        nc.vector.tensor_copy(
                            ys[:, 512 * half : 512 * (half + 1)], yp[:]
                        )
                    if split:
                        nc.sync.dma_start(
                            y[
                                t0 + 128 * c : t0 + 128 * (c + 1),
                                512 * half : 512 * (half + 1),
                            ],
                            ys[:, 512 * half : 512 * (half + 1)],
                        )
                if not split:
                    nc.sync.dma_start(
                        y[t0 + 128 * c : t0 + 128 * (c + 1), :], ys[:]
                    )

            def outproj_units(ops_d):
                """Partial output projection for a finished tile, as
                single-matmul units; reads the normalized O^T lazily so it
                can be scheduled before the norm_unit closure has run at
                emission time."""
                ys_d = {}

                def mk(c, half):
                    def run():
                        otn, t0 = ops_d["otn"]
                        if half == 0:
                            ys_d[c] = ysp.tile(
                                [128, E], f16, tag="ys", name=f"ys_{t0}_{c}"
                            )
                        ys = ys_d[c]
                        yp = mmp.tile(
                            [128, 512], f32, tag="mm", name=f"yp_{t0}_{c}_{half}"
                        )
                        nc.tensor.matmul(
                            yp[:],
                            otn[:, 128 * c : 128 * (c + 1)],
                            wo_sb[:, 512 * half : 512 * (half + 1)],
                            start=True,
                            stop=True,
                        )
                        nc.vector.tensor_copy(
                            ys[:, 512 * half : 512 * (half + 1)], yp[:]
                        )
                        if half == 1:
                            nc.sync.dma_start(
                                y[t0 + 128 * c : t0 + 128 * (c + 1), :], ys[:]
                            )
                    return run

                return [mk(c, h) for c in range(4) for h in range(2)]

            def norm_unit(ops, t0):
                """Denominator broadcast -> reciprocal -> normalized O^T.
                Runs as an early filler of the NEXT tile so the PE never
                waits on the reciprocal chain. The rd copies are emitted at
                the owning tile's end (vector only, no PE)."""
                def run():
                    rd0, rd1 = ops["rd"]
                    bps = mmp.tile([128, TQ], f32, tag="mm", name=f"bps_{t0}")
                    nc.tensor.matmul(
                        bps[:], sel[0:1, 0:128], rd0[:], start=True, stop=False
                    )
                    nc.tensor.matmul(
                        bps[:], sel[0:1, 128:256], rd1[:], start=False, stop=True
                    )
                    bb = bbp.tile([128, TQ], f32, tag="bb", name=f"bb_{t0}")
                    nc.vector.reciprocal_approx_fast(bb[:], bps[:])
                    otn = otnp.tile([128, TQ], f16, tag="otn", name=f"otn_{t0}")
                    nc.vector.tensor_mul(
                        otn[0:64, :], ops[0][0:64, :], bb[0:64, :]
                    )
                    nc.vector.tensor_mul(
                        otn[64:128, :], ops[1][0:64, :], bb[64:128, :]
                    )
                    ops["otn"] = (otn, t0)
                return run

            # ---- main pipeline ----
            kt_tiles = []
            v_tiles = []

            q0, kv0, out0 = qkv_units(xts0)
            for u in q0 + kv0:
                u()
            cur = out0
            kv_pending = []  # K/V units of the current tile j (early chunks)
            # output projections are deferred to the late, scalar-bound tiles
            OPSCHED = {5: [0], 6: [1], 7: [2, 3, 4, 5, 6]}
            tile_ops = {}  # j -> ops dict (op tiles, rd, otn)

            for j in range(NJ):
                t0 = TQ * j
                qt = cur["qt"]

                nchunks = 4 * j + 4
                slots = {}

                def place(units, lo, hi):
                    # spread units over chunk indices [lo, hi] (inclusive)
                    if not units:
                        return
                    lo = max(0, min(lo, nchunks - 1))
                    hi = max(lo, min(hi, nchunks - 1))
                    span = hi - lo + 1
                    for i, u in enumerate(units):
                        g = lo + (i * span) // len(units)
                        slots.setdefault(g, []).append(u)

                # normalization of the previous tile: first chunk
                if j > 0:
                    place([norm_unit(tile_ops[j - 1], TQ * (j - 1))], 0, 0)
                # this tile's K/V: must land before the diagonal chunks (4j)
                place(kv_pending, 0, 4 * j - 1)
                fillers = []
                nxt = None
                if j + 1 < NJ:
                    xts = load_xt(j + 1)
                    qu, kvu, nxt = qkv_units(xts)
                    fillers += qu
                    kv_pending = kvu
                else:
                    kv_pending = []
                for i in OPSCHED.get(j, ()):
                    fillers += outproj_units(tile_ops[i])
                place(fillers, 2, nchunks - 1)

                class OpsDict(dict):
                    pass

                ops = OpsDict()
                ops[0] = opp.tile([65, TQ], f32, tag="op", name="op0")
                ops[1] = opp.tile([65, TQ], f32, tag="op", name="op1")
                tile_ops[j] = ops

                def emit_scores(g):
                    jj, c = divmod(g, 4)
                    r = g - 4 * j
                    col0 = 128 * r if r >= 0 else 0
                    st = stp.tile([128, 2 * TQ], f32, tag="st")
                    for h in range(2):
                        nc.tensor.matmul(
                            st[:, TQ * h + col0 : TQ * h + TQ],
                            kt_tiles[jj][64 * h : 64 * h + 64, 128 * c : 128 * (c + 1)],
                            qt[64 * h : 64 * h + 64, col0:TQ],
                            start=True,
                            stop=True,
                        )
                    return st, col0

                def emit_exp(st, col0, g):
                    ex = exp_p.tile([128, 2 * TQ], f16, tag="ex")
                    st3 = st.rearrange("p (h n) -> p h n", h=2)
                    ex3 = ex.rearrange("p (h n) -> p h n", h=2)
                    nc.scalar.activation(
                        ex3[:, :, col0:TQ], st3[:, :, col0:TQ], EXP, scale=0.125
                    )
                    r = g - 4 * j
                    if r >= 0:
                        for h in range(2):
                            nc.gpsimd.tensor_mul(
                                ex[:, TQ * h + col0 : TQ * h + col0 + 128],
                                ex[:, TQ * h + col0 : TQ * h + col0 + 128],
                                mask[:],
                            )
                    return ex

                def emit_av(ex, col0, g):
                    jj, c = divmod(g, 4)
                    for h in range(2):
                        nc.tensor.matmul(
                            ops[h][:, col0:TQ],
                            v_tiles[jj][:, 130 * c + 65 * h : 130 * c + 65 * h + 65],
                            ex[:, TQ * h + col0 : TQ * h + TQ],
                            start=(g == 0),
                            stop=(g == nchunks - 1),
                            skip_group_check=True,
                        )

                # software-pipelined chunk stream: S(g+1) issues before A(g)
                pend = None
                for g in range(nchunks):
                    st, col0 = emit_scores(g)
                    ex = emit_exp(st, col0, g)
                    for u in slots.get(g, ()):
                        u()
                    if pend is not None:
                        emit_av(*pend)
                    pend = (ex, col0, g)
                emit_av(*pend)

                # grab the denominator rows; the rest of the normalization
                # runs as the next tile's first filler
                rd0 = rdp.tile([1, TQ], f32r, tag="rd", name="rd0")
                rd1 = rdp.tile([1, TQ], f32r, tag="rd", name="rd1")
                with nc.allow_low_precision(reason="f32r rounding of denom"):
                    nc.vector.tensor_copy(rd0[:], ops[0][64:65, :])
                    nc.vector.tensor_copy(rd1[:], ops[1][64:65, :])
                ops["rd"] = (rd0, rd1)
                cur = nxt

            # ---- tail: last tile's normalization + projection, pipelined
            # per 128-token chunk so casts/DMAs overlap the matmuls
            ops = tile_ops[NJ - 1]
            t0 = TQ * (NJ - 1)
            rd0, rd1 = ops["rd"]
            bps = mmp.tile([128, TQ], f32, tag="mm", name="bps_last")
            nc.tensor.matmul(bps[:], sel[0:1, 0:128], rd0[:], start=True, stop=False)
            nc.tensor.matmul(
                bps[:], sel[0:1, 128:256], rd1[:], start=False, stop=True
            )
            bb = bbp.tile([128, TQ], f32, tag="bb", name="bb_last")
            nc.vector.reciprocal_approx_fast(bb[:], bps[:])
            otn = otnp.tile([128, TQ], f16, tag="otn", name="otn_last")
            for c in range(4):
                sl = slice(128 * c, 128 * (c + 1))
                nc.vector.tensor_mul(otn[0:64, sl], ops[0][0:64, sl], bb[0:64, sl])
                nc.vector.tensor_mul(
                    otn[64:128, sl], ops[1][0:64, sl], bb[64:128, sl]
                )
                proj_chunk(otn, t0, c, split=True)

    nc.compile()
    return nc


def _prep_inputs(x, w_qkv, b_qkv, w_out, b_out):
    x = np.asarray(x, dtype=np.float32).reshape(T, E)
    w_qkv = np.asarray(w_qkv, dtype=np.float32)
    b_qkv = np.asarray(b_qkv, dtype=np.float32)
    w_out = np.asarray(w_out, dtype=np.float32)
    b_out = np.asarray(b_out, dtype=np.float32)

    xT = np.ascontiguousarray(x.T).astype(np.float16)
    mask = np.triu(np.ones((128, 128), dtype=np.float16))
    ident = np.eye(128, dtype=np.float16)
    sel = np.zeros((1, 256), dtype=np.float32)
    sel[0, 0:64] = 1.0
    sel[0, 192:256] = 1.0

    in_maps = []
    for cidx in range(NCORES):
        lo, hi = 128 * cidx, 128 * (cidx + 1)
        wq = w_qkv[:, lo:hi]
        wk = w_qkv[:, E + lo : E + hi]
        wv = w_qkv[:, 2 * E + lo : 2 * E + hi]
        wqkv_c = np.ascontiguousarray(
            np.concatenate([wq, wk, wv], axis=1)
        ).astype(np.float16)
        in_maps.append(
            {
                "xT": xT,
                "wqkv": wqkv_c,
                "bq": np.ascontiguousarray(b_qkv[lo:hi]),
                "wo": np.ascontiguousarray(w_out[lo:hi, :]).astype(np.float16),
                "sel": sel,
                "mask": mask,
                "ident": ident,
            }
        )
    # host-side constant: b_out plus the exact b_v contribution
    b_v = b_qkv[2 * E : 3 * E]
    const_row = b_out + b_v @ w_out
    return in_maps, const_row


def kernel(x, w_qkv, b_qkv, w_out, b_out):
    global LAST_RESULTS
    from concourse.bass_utils import run_bass_kernel_spmd

    if "nc" not in _CACHE:
        _CACHE["nc"] = _build()
    nc = _CACHE["nc"]

    in_maps, const_row = _prep_inputs(x, w_qkv, b_qkv, w_out, b_out)
    res = run_bass_kernel_spmd(nc, in_maps, core_ids=list(range(NCORES)))
    LAST_RESULTS = res

    out = np.zeros((T, E), dtype=np.float32)
    for r in res.results:
        out += r["y"].astype(np.float32)
    out += const_row[None, :].astype(np.float32)
    return out.reshape(1, T, E)
